# revision 9
# baseline (speedup 1.0000x reference)
"""GCLayer GNN message-passing kernel for 8 Trainium2 NeuronCores (Bass/Tile).

Strategy: destination-sharded edge parallelism — no collectives.
- Nodes padded to NPAD = 50176 and split into 8 shards of SH = 6272.
- Core k owns node shard k: it receives node inputs in a rolled order
  (its shard first), computes z = x@W_lin + silu(temb)@Wt + b for ALL
  nodes, builds gather tables a = z@(W_lin1@We1_top) (shard only, +be1
  handled via ACT bias) and b = z@(W_lin1@We1_bot) (all nodes) in DRAM,
  and h_shard = z_shard@W_lin1.
- Edges are routed on the host to the core owning their destination row,
  sorted by 128-node window, and padded to a schedule (chunks per window
  per col-half) that is identical across cores, so one SPMD program works.
- Per 128-edge chunk: transposed bf16 dma_gather of a[row], b[col];
  z1 = a+b; s1 = silu(z1+be1); attention logits via N=1 matmuls with
  p = We1_top^-1 @ wa_top, q = We1_bot^-1 @ wa_bot (host-solved);
  mT = We2-matmul; msgT = silu(mT + be2); PE transpose to msg-normal;
  scatter into a per-window PSUM accumulator via a one-hot matmul whose
  one-hot is fused with att*edge_mask on the vector engine.
- Post: out = h + silu([h,agg]@Wn1 + bn1)@Wn2 + bn2, PE-transposed and
  written per-shard; host reassembles and applies node_mask.

Hardcoded problem: N=50000, E=800000, D=128, n_cores=8.
"""
import math
from dataclasses import dataclass, field

import numpy as np
import ml_dtypes

BF = ml_dtypes.bfloat16
F32 = np.float32
P = 128


@dataclass
class Cfg:
    N: int = 50000
    E: int = 800000
    NCORES: int = 8
    NPAD: int = 50176          # multiple of NCORES*128
    HALF: int = 32768          # int16 split point for the b-table gather
    TILE: int = 512

    @property
    def SH(self):
        return self.NPAD // self.NCORES

    @property
    def NW(self):
        return self.SH // P


@dataclass
class Meta:
    """Compile-time schedule + per-core data."""
    cntA: list = field(default_factory=list)   # chunks per window, col-half A
    cntB: list = field(default_factory=list)   # chunks per window, col-half B
    nch: int = 0                               # total chunks per core
    in_maps: list = field(default_factory=list)
    shared: dict = field(default_factory=dict)


def _silu(x):
    return x / (1.0 + np.exp(-x))


def _wrap_idx(arr):
    """[L] int16 -> [128, L//16] wrapped (i -> [i%16, i//16]) and replicated."""
    L = arr.shape[0]
    wr = arr.reshape(L // 16, 16).T.copy()
    return np.tile(wr, (8, 1))


def host_prep(cfg, x, edges, node_mask, edge_mask, temb,
              W_lin, b_lin, W_lin1, Wt, bt,
              W_att, b_att, We1, be1, We2, be2,
              Wn1, bn1, Wn2, bn2):
    D = P
    N, NPAD, SH, NW, NC = cfg.N, cfg.NPAD, cfg.SH, cfg.NW, cfg.NCORES

    # ---- shared weights
    W_lin64 = np.asarray(W_lin, np.float64)
    W_lin1_64 = np.asarray(W_lin1, np.float64)
    We1_64 = np.asarray(We1, np.float64)
    W_att64 = np.asarray(W_att, np.float64)
    Ga = (W_lin1_64 @ We1_64[:D]).astype(BF)
    Gb = (W_lin1_64 @ We1_64[D:]).astype(BF)
    pvec = np.linalg.solve(We1_64[:D], W_att64[:D]).astype(BF)       # [D,1]
    qvec = np.linalg.solve(We1_64[D:], W_att64[D:]).astype(BF)
    shared = dict(
        w_lin=np.asarray(W_lin, F32).astype(BF),
        wt=np.asarray(Wt, F32).astype(BF),
        blt=(np.asarray(b_lin, F32) + np.asarray(bt, F32)).reshape(D, 1),
        ga=Ga, gb=Gb,
        w_lin1=np.asarray(W_lin1, F32).astype(BF),
        pvec=pvec, qvec=qvec,
        we2=np.asarray(We2, F32).astype(BF),
        be1c=np.asarray(be1, F32).reshape(D, 1),
        be2c=np.asarray(be2, F32).reshape(D, 1),
        wn1h=np.asarray(Wn1, F32)[:D].astype(BF),
        wn1a=np.asarray(Wn1, F32)[D:].astype(BF),
        wn2=np.asarray(Wn2, F32).astype(BF),
        bn1c=np.asarray(bn1, F32).reshape(D, 1),
        bn2c=np.asarray(bn2, F32).reshape(D, 1),
        battc=np.full((D, 1), float(np.asarray(b_att).reshape(-1)[0]), F32),
        ident_bf=np.eye(P, dtype=F32).astype(BF),
        ident_f32=np.eye(P, dtype=F32),
        iota_row=np.tile(np.arange(P, dtype=F32), (P, 1)),
    )
    b_att_f = float(np.asarray(b_att).reshape(-1)[0])

    # ---- node features: pad + transpose + bf16, per-core roll
    xT = np.zeros((NPAD, D), F32)
    xT[:N] = np.asarray(x, F32)
    xT = np.ascontiguousarray(xT.T).astype(BF)        # [D, NPAD]
    tT = np.zeros((NPAD, D), F32)
    tT[:N] = np.asarray(temb, F32)
    tT = np.ascontiguousarray(tT.T).astype(BF)

    # ---- edge routing
    row = np.asarray(edges[0], np.int64)
    col = np.asarray(edges[1], np.int64)
    em = np.asarray(edge_mask, F32).reshape(-1)
    shard = row // SH

    per_core = []
    cA = np.zeros((NC, NW), np.int64)
    cB = np.zeros((NC, NW), np.int64)
    for k in range(NC):
        m = shard == k
        r = row[m] - k * SH
        c = col[m]
        e = em[m]
        bcol = (c - k * SH) % NPAD
        w = r // P
        half = (bcol >= cfg.HALF).astype(np.int64)
        order = np.lexsort((half, w))
        r, bcol, e, w, half = r[order], bcol[order], e[order], w[order], half[order]
        cnt = np.bincount(w * 2 + half, minlength=2 * NW)
        cA[k] = cnt[0::2]
        cB[k] = cnt[1::2]
        per_core.append((r, bcol, e, w, half))

    chA = [int(math.ceil(cA[:, w].max() / P)) for w in range(NW)]
    chB = [int(math.ceil(cB[:, w].max() / P)) for w in range(NW)]
    nch = sum(chA) + sum(chB)
    TE = nch * P

    in_maps = []
    for k in range(NC):
        r, bcol, e, w, half = per_core[k]
        aidx = np.zeros(TE, np.int16)
        bidx = np.zeros(TE, np.int16)
        lrow = np.zeros(TE, F32)
        emk = np.zeros(TE, F32)
        # fill padded runs window by window
        pos_src = 0
        pos_dst = 0
        cntk = np.bincount(w * 2 + half, minlength=2 * NW)
        for wi in range(NW):
            for h, ch in ((0, chA[wi]), (1, chB[wi])):
                n_real = int(cntk[wi * 2 + h])
                L = ch * P
                if n_real:
                    sl_src = slice(pos_src, pos_src + n_real)
                    sl_dst = slice(pos_dst, pos_dst + n_real)
                    aidx[sl_dst] = r[sl_src].astype(np.int16)
                    bc = bcol[sl_src]
                    bidx[sl_dst] = (bc - (cfg.HALF if h else 0)).astype(np.int16)
                    lrow[sl_dst] = (r[sl_src] - wi * P).astype(F32)
                    emk[sl_dst] = e[sl_src]
                    pos_src += n_real
                pos_dst += L
        assert pos_src == r.shape[0]
        assert pos_dst == TE
        im = dict(shared)
        im["x_t"] = np.ascontiguousarray(np.roll(xT, -k * SH, axis=1))
        im["temb_t"] = np.ascontiguousarray(np.roll(tT, -k * SH, axis=1))
        im["aidx"] = _wrap_idx(aidx)
        im["bidx"] = _wrap_idx(bidx)
        im["lrow"] = np.ascontiguousarray(lrow.reshape(nch, P).T)
        im["emk"] = np.ascontiguousarray(emk.reshape(nch, P).T)
        in_maps.append(im)

    meta = Meta(cntA=chA, cntB=chB, nch=nch, in_maps=in_maps, shared=shared)
    meta.b_att = b_att_f
    return meta


# ---------------------------------------------------------------------------
# Device program
# ---------------------------------------------------------------------------

def build_nc(cfg, meta):
    import concourse.bacc as bacc
    import concourse.tile as tile
    import concourse.mybir as mybir

    D = P
    NPAD, SH, NW = cfg.NPAD, cfg.SH, cfg.NW
    TILE = cfg.TILE
    nch = meta.nch
    TE = nch * P
    dt = mybir.dt
    AF = mybir.ActivationFunctionType
    ALU = mybir.AluOpType

    nc = bacc.Bacc("TRN2", target_bir_lowering=False, debug=False,
                   num_devices=cfg.NCORES)

    def din(name, shape, dtype):
        return nc.dram_tensor(name, shape, dtype, kind="ExternalInput")

    x_t = din("x_t", [D, NPAD], dt.bfloat16)
    temb_t = din("temb_t", [D, NPAD], dt.bfloat16)
    w_lin = din("w_lin", [D, D], dt.bfloat16)
    wt = din("wt", [D, D], dt.bfloat16)
    blt = din("blt", [D, 1], dt.float32)
    ga = din("ga", [D, D], dt.bfloat16)
    gb = din("gb", [D, D], dt.bfloat16)
    w_lin1 = din("w_lin1", [D, D], dt.bfloat16)
    pvec = din("pvec", [D, 1], dt.bfloat16)
    qvec = din("qvec", [D, 1], dt.bfloat16)
    we2 = din("we2", [D, D], dt.bfloat16)
    be1c = din("be1c", [D, 1], dt.float32)
    be2c = din("be2c", [D, 1], dt.float32)
    wn1h = din("wn1h", [D, D], dt.bfloat16)
    wn1a = din("wn1a", [D, D], dt.bfloat16)
    wn2 = din("wn2", [D, D], dt.bfloat16)
    bn1c = din("bn1c", [D, 1], dt.float32)
    battc = din("battc", [D, 1], dt.float32)
    bn2c = din("bn2c", [D, 1], dt.float32)
    ident_bf = din("ident_bf", [P, P], dt.bfloat16)
    ident_f32 = din("ident_f32", [P, P], dt.float32)
    iota_row = din("iota_row", [P, P], dt.float32)
    aidx_d = din("aidx", [P, TE // 16], dt.int16)
    bidx_d = din("bidx", [P, TE // 16], dt.int16)
    lrow_d = din("lrow", [P, nch], dt.float32)
    emk_d = din("emk", [P, nch], dt.float32)

    out_d = nc.dram_tensor("out", [SH, D], dt.float32, kind="ExternalOutput")

    # node-stage column tiles: shard first (multiples of 128), then the rest
    tiles = []
    s = 0
    while s < SH:
        w = min(TILE, SH - s)
        tiles.append((s, w))
        s += w
    while s < NPAD:
        w = min(TILE, NPAD - s)
        tiles.append((s, w))
        s += w

    with tile.TileContext(nc) as tc:
        with (
            tc.tile_pool(name="cst", bufs=1) as cst,
            tc.tile_pool(name="pers", bufs=1) as pers,
            tc.tile_pool(name="sb", bufs=3) as sb,
            tc.tile_pool(name="gth", bufs=2) as gth,
            tc.tile_pool(name="ps", bufs=2, space="PSUM") as ps,
            tc.tile_pool(name="ps1", bufs=1, space="PSUM") as ps1,
            tc.tile_pool(name="ps2", bufs=2, space="PSUM") as ps2,
            tc.tile_pool(name="dram", bufs=1, space="DRAM") as dpool,
        ):
            # ---- constants to SBUF
            def ld(ap_, shape, dtype):
                t = cst.tile(shape, dtype, tag=f"c_{ap_.name}")
                nc.sync.dma_start(t[:], ap_.ap())
                return t

            w_lin_c = ld(w_lin, [D, D], dt.bfloat16)
            wt_c = ld(wt, [D, D], dt.bfloat16)
            blt_c = ld(blt, [D, 1], dt.float32)
            ga_c = ld(ga, [D, D], dt.bfloat16)
            gb_c = ld(gb, [D, D], dt.bfloat16)
            w_lin1_c = ld(w_lin1, [D, D], dt.bfloat16)
            p_c = ld(pvec, [D, 1], dt.bfloat16)
            q_c = ld(qvec, [D, 1], dt.bfloat16)
            we2_c = ld(we2, [D, D], dt.bfloat16)
            be1_c = ld(be1c, [D, 1], dt.float32)
            be2_c = ld(be2c, [D, 1], dt.float32)
            wn1h_c = ld(wn1h, [D, D], dt.bfloat16)
            wn1a_c = ld(wn1a, [D, D], dt.bfloat16)
            wn2_c = ld(wn2, [D, D], dt.bfloat16)
            bn1_c = ld(bn1c, [D, 1], dt.float32)
            batt_c = ld(battc, [D, 1], dt.float32)
            bn2_c = ld(bn2c, [D, 1], dt.float32)
            identb_c = ld(ident_bf, [P, P], dt.bfloat16)
            identf_c = ld(ident_f32, [P, P], dt.float32)
            iota_c = ld(iota_row, [P, P], dt.float32)
            aidx_c = ld(aidx_d, [P, TE // 16], dt.int16)
            bidx_c = ld(bidx_d, [P, TE // 16], dt.int16)
            lrow_c = ld(lrow_d, [P, nch], dt.float32)
            emk_c = ld(emk_d, [P, nch], dt.float32)

            # ---- persistent SBUF
            hT_f32 = pers.tile([D, SH], dt.float32)
            hT_bf = pers.tile([D, SH], dt.bfloat16)
            aggT_bf = pers.tile([D, SH], dt.bfloat16)

            # ---- DRAM gather tables
            atab = dpool.tile([SH, D], dt.bfloat16)
            btab = dpool.tile([NPAD, D], dt.bfloat16)

            # ================= node stage =================
            for (s0, wd) in tiles:
                xt = sb.tile([D, TILE], dt.bfloat16, tag="xt")
                nc.sync.dma_start(xt[:, :wd], x_t.ap()[:, s0:s0 + wd])
                tt = sb.tile([D, TILE], dt.bfloat16, tag="tt")
                nc.sync.dma_start(tt[:, :wd], temb_t.ap()[:, s0:s0 + wd])
                st = sb.tile([D, TILE], dt.bfloat16, tag="st")
                nc.scalar.activation(out=st[:, :wd], in_=tt[:, :wd], func=AF.Silu)
                zp = ps.tile([D, TILE], dt.float32, tag="pbig")
                nc.tensor.matmul(out=zp[:, :wd], lhsT=w_lin_c[:], rhs=xt[:, :wd],
                                 start=True, stop=False)
                nc.tensor.matmul(out=zp[:, :wd], lhsT=wt_c[:], rhs=st[:, :wd],
                                 start=False, stop=True)
                zt = sb.tile([D, TILE], dt.bfloat16, tag="zt")
                nc.vector.tensor_scalar_add(zt[:, :wd], zp[:, :wd], blt_c[:])

                in_shard = s0 + wd <= SH
                if in_shard:
                    hp = ps.tile([D, TILE], dt.float32, tag="pbig")
                    nc.tensor.matmul(out=hp[:, :wd], lhsT=w_lin1_c[:],
                                     rhs=zt[:, :wd], start=True, stop=True)
                    nc.vector.tensor_copy(hT_f32[:, s0:s0 + wd], hp[:, :wd])
                    nc.vector.tensor_copy(hT_bf[:, s0:s0 + wd], hp[:, :wd])

                for c in range(wd // P):
                    zc = zt[:, c * P:(c + 1) * P]
                    bp = ps1.tile([P, P], dt.float32, tag="psmall")
                    nc.tensor.matmul(out=bp[:], lhsT=zc, rhs=gb_c[:],
                                     start=True, stop=True)
                    bs = sb.tile([P, P], dt.bfloat16, tag="bs")
                    nc.vector.tensor_copy(bs[:], bp[:])
                    nc.sync.dma_start(btab[s0 + c * P:s0 + (c + 1) * P, :], bs[:])
                    if in_shard:
                        ap_ = ps1.tile([P, P], dt.float32, tag="psmall")
                        nc.tensor.matmul(out=ap_[:], lhsT=zc, rhs=ga_c[:],
                                         start=True, stop=True)
                        as_ = sb.tile([P, P], dt.bfloat16, tag="as_")
                        nc.vector.tensor_copy(as_[:], ap_[:])
                        nc.sync.dma_start(
                            atab[s0 + c * P:s0 + (c + 1) * P, :], as_[:])

            # ================= edge stage =================
            gmax = max(max(meta.cntA), max(meta.cntB))
            ci = 0      # global chunk index
            for w in range(NW):
                gtot = meta.cntA[w] + meta.cntB[w]
                aggp = ps2.tile([D, P], dt.float32, tag="aggp")
                if gtot == 0:
                    nc.vector.memset(aggT_bf[:, w * P:(w + 1) * P], 0.0)
                    continue
                done = 0
                for half in (0, 1):
                    G = meta.cntA[w] if half == 0 else meta.cntB[w]
                    if G == 0:
                        continue
                    btab_v = btab[:cfg.HALF, :] if half == 0 else btab[cfg.HALF:, :]
                    # process in sub-batches of <=4 chunks (dma_gather HW
                    # limit: num_idxs < 1024 in transpose mode)
                    for b0 in range(0, G, 4):
                        gb4 = min(4, G - b0)
                        L = gb4 * P
                        cib = ci + b0
                        gaT = gth.tile([P, 1, 4 * P], dt.bfloat16, tag="gaT")
                        nc.gpsimd.dma_gather(
                            out_ap=gaT[:, :, :L], in_ap=atab[:, :],
                            idxs_ap=aidx_c[:, cib * 8:(cib + gb4) * 8],
                            num_idxs=L, num_idxs_reg=L, elem_size=D,
                            transpose=True)
                        gbT = gth.tile([P, 1, 4 * P], dt.bfloat16, tag="gbT")
                        nc.gpsimd.dma_gather(
                            out_ap=gbT[:, :, :L], in_ap=btab_v,
                            idxs_ap=bidx_c[:, cib * 8:(cib + gb4) * 8],
                            num_idxs=L, num_idxs_reg=L, elem_size=D,
                            transpose=True)
                        z1 = sb.tile([P, 4 * P], dt.bfloat16, tag="z1")
                        nc.vector.tensor_add(z1[:, :L], gaT[:, 0, :L],
                                             gbT[:, 0, :L])
                        s1 = sb.tile([P, 4 * P], dt.bfloat16, tag="s1")
                        nc.scalar.activation(out=s1[:, :L], in_=z1[:, :L],
                                             func=AF.Silu, bias=be1_c[:])
                        # attention logits, one column per chunk
                        lp = ps1.tile([P, 4], dt.float32, tag="plog")
                        for c in range(gb4):
                            nc.tensor.matmul(out=lp[:, c:c + 1],
                                             lhsT=gaT[:, 0, c * P:(c + 1) * P],
                                             rhs=p_c[:], start=True, stop=False)
                            nc.tensor.matmul(out=lp[:, c:c + 1],
                                             lhsT=gbT[:, 0, c * P:(c + 1) * P],
                                             rhs=q_c[:], start=False, stop=True)
                        att = sb.tile([P, 4], dt.float32, tag="att")
                        nc.scalar.activation(out=att[:, :gb4], in_=lp[:, :gb4],
                                             func=AF.Sigmoid, bias=batt_c[:])
                        attm = sb.tile([P, 4], dt.float32, tag="attm")
                        nc.vector.tensor_mul(attm[:, :gb4], att[:, :gb4],
                                             emk_c[:, cib:cib + gb4])
                        # message MLP second layer + transpose + scatter
                        mp = ps.tile([P, 4 * P], dt.float32, tag="pbig")
                        nc.tensor.matmul(out=mp[:, :L], lhsT=we2_c[:],
                                         rhs=s1[:, :L], start=True, stop=True)
                        msgT = sb.tile([P, 4 * P], dt.bfloat16, tag="msgT")
                        nc.scalar.activation(out=msgT[:, :L], in_=mp[:, :L],
                                             func=AF.Silu, bias=be2_c[:])
                        tp = ps1.tile([P, 4 * P], dt.bfloat16, tag="ptp")
                        for c4 in range(gb4):
                            nc.tensor.transpose(
                                out=tp[:, c4 * P:(c4 + 1) * P],
                                in_=msgT[:, c4 * P:(c4 + 1) * P],
                                identity=identb_c[:])
                        msgN = sb.tile([P, 4 * P], dt.bfloat16, tag="msgN")
                        nc.vector.tensor_copy(msgN[:, :L], tp[:, :L])
                        for c4 in range(gb4):
                            oh = sb.tile([P, P], dt.bfloat16, tag="oh")
                            nc.vector.tensor_scalar(
                                out=oh[:], in0=iota_c[:],
                                scalar1=lrow_c[:, cib + c4:cib + c4 + 1],
                                scalar2=attm[:, c4:c4 + 1],
                                op0=ALU.is_equal, op1=ALU.mult)
                            nc.tensor.matmul(
                                out=aggp[:], lhsT=msgN[:, c4 * P:(c4 + 1) * P],
                                rhs=oh[:], start=(done == 0),
                                stop=(done == gtot - 1))
                            done += 1
                    ci += G
                nc.vector.tensor_copy(aggT_bf[:, w * P:(w + 1) * P], aggp[:])

            # ================= post stage =================
            s = 0
            while s < SH:
                wd = min(TILE, SH - s)
                yp = ps.tile([D, TILE], dt.float32, tag="pbig")
                nc.tensor.matmul(out=yp[:, :wd], lhsT=wn1h_c[:],
                                 rhs=hT_bf[:, s:s + wd], start=True, stop=False)
                nc.tensor.matmul(out=yp[:, :wd], lhsT=wn1a_c[:],
                                 rhs=aggT_bf[:, s:s + wd], start=False, stop=True)
                y1 = sb.tile([D, TILE], dt.bfloat16, tag="y1")
                nc.scalar.activation(out=y1[:, :wd], in_=yp[:, :wd],
                                     func=AF.Silu, bias=bn1_c[:])
                y2p = ps.tile([D, TILE], dt.float32, tag="pbig")
                nc.tensor.matmul(out=y2p[:, :wd], lhsT=wn2_c[:],
                                 rhs=y1[:, :wd], start=True, stop=True)
                o1 = sb.tile([D, TILE], dt.float32, tag="o1")
                nc.vector.tensor_scalar_add(o1[:, :wd], y2p[:, :wd], bn2_c[:])
                o2 = sb.tile([D, TILE], dt.float32, tag="o2")
                nc.vector.tensor_add(o2[:, :wd], o1[:, :wd], hT_f32[:, s:s + wd])
                for c in range(wd // P):
                    top = ps1.tile([P, P], dt.float32, tag="psmall")
                    nc.tensor.transpose(out=top[:], in_=o2[:, c * P:(c + 1) * P],
                                        identity=identf_c[:])
                    os_ = sb.tile([P, P], dt.float32, tag="os_")
                    nc.vector.tensor_copy(os_[:], top[:])
                    nc.sync.dma_start(out_d.ap()[s + c * P:s + (c + 1) * P, :],
                                      os_[:])
                s += wd

    nc.compile()
    return nc


# ---------------------------------------------------------------------------
# Entry point
# ---------------------------------------------------------------------------

_STATE = {}


def kernel(x, edges, node_mask, edge_mask, temb,
           W_lin, b_lin, W_lin1, Wt, bt,
           W_att, b_att, We1, be1, We2, be2,
           Wn1, bn1, Wn2, bn2):
    from concourse import bass_utils

    cfg = Cfg()
    meta = host_prep(cfg, x, edges, node_mask, edge_mask, temb,
                     W_lin, b_lin, W_lin1, Wt, bt,
                     W_att, b_att, We1, be1, We2, be2,
                     Wn1, bn1, Wn2, bn2)
    nc = build_nc(cfg, meta)
    _STATE.update(cfg=cfg, meta=meta, nc=nc)
    res = bass_utils.run_bass_kernel_spmd(
        nc, meta.in_maps, core_ids=list(range(cfg.NCORES)))
    _STATE["res"] = res
    out = np.concatenate([res.results[k]["out"] for k in range(cfg.NCORES)],
                         axis=0)[:cfg.N]
    out = out.astype(F32) * np.asarray(node_mask, F32)
    return out


def run_traced():
    """Re-run the already-built kernel with NTFF profiling; returns results
    carrying exec_time_ns (test harness helper, not used by the grader)."""
    from concourse import bass_utils
    cfg, meta, nc = _STATE["cfg"], _STATE["meta"], _STATE["nc"]
    return bass_utils.run_bass_kernel_spmd(
        nc, meta.in_maps, core_ids=list(range(cfg.NCORES)), trace=True)
